# revision 2
# baseline (speedup 1.0000x reference)
"""Trainium2 Bass kernel: per-node mean over gathered hyperedge embeddings.

out[n, :] = mean_k table[idx[n, k], :]   (idx: [100000, 32], table: [500000, 128])

Strategy: shard nodes across 8 NeuronCores data-parallel; replicate the
embedding table. Each core runs tiles of 128 nodes: one indirect DMA
gathers the 128x32 embedding rows (one node per SBUF partition, its 32
rows along the free dim), DVE reduces over the 32 rows, result is stored.
The 1/32 mean scale is folded into the table on the host (exact in fp32).
"""

import numpy as np

import concourse.bass as bass
import concourse.tile as tile
from concourse import bacc, mybir
from concourse.bass_utils import run_bass_kernel_spmd

P = 128
N_EDGES = 500000
EMBED = 128
N_NODES = 100000
DEGREE = 32
N_CORES = 8
NODES_PER_CORE = N_NODES // N_CORES  # 12500
TILES = 98  # ceil(12500 / 128)
PAD_NODES = TILES * P  # 12544

_prog_cache = {}


def _build():
    if "nc" in _prog_cache:
        return _prog_cache["nc"]
    nc = bacc.Bacc(
        "TRN2",
        target_bir_lowering=False,
        debug=False,
        enable_asserts=False,
        num_devices=N_CORES,
    )
    table = nc.dram_tensor(
        "table", [N_EDGES, EMBED], mybir.dt.float32, kind="ExternalInput"
    ).ap()
    idx = nc.dram_tensor(
        "idx", [PAD_NODES, DEGREE], mybir.dt.int32, kind="ExternalInput"
    ).ap()
    out = nc.dram_tensor(
        "out", [PAD_NODES, EMBED], mybir.dt.float32, kind="ExternalOutput"
    ).ap()

    with tile.TileContext(nc) as tc:
        with (
            tc.tile_pool(name="idxp", bufs=4) as idxp,
            tc.tile_pool(name="gath", bufs=4) as gathp,
            tc.tile_pool(name="outp", bufs=4) as outp,
        ):
            for t in range(TILES):
                idx_t = idxp.tile([P, DEGREE], mybir.dt.int32)
                nc.sync.dma_start(out=idx_t[:], in_=idx[t * P : (t + 1) * P, :])
                g = gathp.tile([P, DEGREE * EMBED], mybir.dt.float32)
                # HW indirect DMA consumes exactly one offset per partition;
                # issue one gather per neighbor slot (128 rows each).
                for j in range(DEGREE):
                    nc.gpsimd.indirect_dma_start(
                        out=g[:, j * EMBED : (j + 1) * EMBED],
                        out_offset=None,
                        in_=table[:],
                        in_offset=bass.IndirectOffsetOnAxis(
                            ap=idx_t[:, j : j + 1], axis=0
                        ),
                    )
                o = outp.tile([P, EMBED], mybir.dt.float32)
                gv = g[:].rearrange("p (k d) -> p d k", k=DEGREE)
                nc.vector.tensor_reduce(
                    out=o[:], in_=gv, axis=mybir.AxisListType.X, op=mybir.AluOpType.add
                )
                nc.sync.dma_start(out=out[t * P : (t + 1) * P, :], in_=o[:])
    nc.compile()
    _prog_cache["nc"] = nc
    return nc


def run(embedding_table, node_hyperedges, **spmd_kwargs):
    """Run on 8 cores; returns (full_output, BassKernelResults)."""
    table = np.asarray(embedding_table, dtype=np.float32) * np.float32(1.0 / 32.0)
    table = np.ascontiguousarray(table)
    idx = np.ascontiguousarray(np.asarray(node_hyperedges).astype(np.int32))
    assert table.shape == (N_EDGES, EMBED)
    assert idx.shape == (N_NODES, DEGREE)

    nc = _build()
    in_maps = []
    for c in range(N_CORES):
        shard = idx[c * NODES_PER_CORE : (c + 1) * NODES_PER_CORE]
        pad = np.zeros((PAD_NODES, DEGREE), np.int32)
        pad[:NODES_PER_CORE] = shard
        in_maps.append({"table": table, "idx": pad})

    res = run_bass_kernel_spmd(nc, in_maps, list(range(N_CORES)), **spmd_kwargs)
    out = np.concatenate(
        [res.results[c]["out"][:NODES_PER_CORE] for c in range(N_CORES)], axis=0
    )
    return out, res


def kernel(embedding_table, node_hyperedges):
    out, _ = run(embedding_table, node_hyperedges)
    return out


# revision 3
# speedup vs baseline: 1.0041x; 1.0041x over previous
"""Trainium2 Bass kernel: per-node mean over gathered hyperedge embeddings.

out[n, :] = mean_k table[idx[n, k], :]   (idx: [100000, 32], table: [500000, 128])

Strategy: shard nodes across 8 NeuronCores data-parallel; replicate the
embedding table. Each core runs tiles of 128 nodes: one indirect DMA
gathers the 128x32 embedding rows (one node per SBUF partition, its 32
rows along the free dim), DVE reduces over the 32 rows, result is stored.
The 1/32 mean scale is folded into the table on the host (exact in fp32).
"""

import numpy as np

import concourse.bass as bass
import concourse.tile as tile
from concourse import bacc, mybir
from concourse.bass_utils import run_bass_kernel_spmd

P = 128
N_EDGES = 500000
EMBED = 128
N_NODES = 100000
DEGREE = 32
N_CORES = 8
NODES_PER_CORE = N_NODES // N_CORES  # 12500
TILES = 98  # ceil(12500 / 128)
PAD_NODES = TILES * P  # 12544

_prog_cache = {}


def _build():
    if "nc" in _prog_cache:
        return _prog_cache["nc"]
    nc = bacc.Bacc(
        "TRN2",
        target_bir_lowering=False,
        debug=False,
        enable_asserts=False,
        num_devices=N_CORES,
    )
    table = nc.dram_tensor(
        "table", [N_EDGES, EMBED], mybir.dt.float32, kind="ExternalInput"
    ).ap()
    idx = nc.dram_tensor(
        "idx", [PAD_NODES, DEGREE], mybir.dt.int32, kind="ExternalInput"
    ).ap()
    out = nc.dram_tensor(
        "out", [PAD_NODES, EMBED], mybir.dt.float32, kind="ExternalOutput"
    ).ap()

    with tile.TileContext(nc) as tc:
        with (
            tc.tile_pool(name="idxp", bufs=1) as idxp,
            tc.tile_pool(name="gath", bufs=4) as gathp,
            tc.tile_pool(name="outp", bufs=4) as outp,
        ):
            # Preload ALL node indices once: [12544, 32] -> SBUF [128, 98, 32]
            # (node n = tile t, partition p; idx[t*128+p, j] at [p, t, j]).
            idx_all = idxp.tile([P, TILES, DEGREE], mybir.dt.int32)
            nc.sync.dma_start(
                out=idx_all[:],
                in_=idx[:, :].rearrange("(t p) j -> p t j", p=P),
            )
            for t in range(TILES):
                g = gathp.tile([P, DEGREE * EMBED], mybir.dt.float32)
                # HW indirect DMA consumes exactly one offset per partition;
                # issue one gather per neighbor slot (128 rows each).
                for j in range(DEGREE):
                    nc.gpsimd.indirect_dma_start(
                        out=g[:, j * EMBED : (j + 1) * EMBED],
                        out_offset=None,
                        in_=table[:],
                        in_offset=bass.IndirectOffsetOnAxis(
                            ap=idx_all[:, t, j : j + 1], axis=0
                        ),
                    )
                o = outp.tile([P, EMBED], mybir.dt.float32)
                gv = g[:].rearrange("p (k d) -> p d k", k=DEGREE)
                nc.vector.tensor_reduce(
                    out=o[:], in_=gv, axis=mybir.AxisListType.X, op=mybir.AluOpType.add
                )
                nc.sync.dma_start(out=out[t * P : (t + 1) * P, :], in_=o[:])
    nc.compile()
    _prog_cache["nc"] = nc
    return nc


def run(embedding_table, node_hyperedges, **spmd_kwargs):
    """Run on 8 cores; returns (full_output, BassKernelResults)."""
    table = np.asarray(embedding_table, dtype=np.float32) * np.float32(1.0 / 32.0)
    table = np.ascontiguousarray(table)
    idx = np.ascontiguousarray(np.asarray(node_hyperedges).astype(np.int32))
    assert table.shape == (N_EDGES, EMBED)
    assert idx.shape == (N_NODES, DEGREE)

    nc = _build()
    in_maps = []
    for c in range(N_CORES):
        shard = idx[c * NODES_PER_CORE : (c + 1) * NODES_PER_CORE]
        pad = np.zeros((PAD_NODES, DEGREE), np.int32)
        pad[:NODES_PER_CORE] = shard
        in_maps.append({"table": table, "idx": pad})

    res = run_bass_kernel_spmd(nc, in_maps, list(range(N_CORES)), **spmd_kwargs)
    out = np.concatenate(
        [res.results[c]["out"][:NODES_PER_CORE] for c in range(N_CORES)], axis=0
    )
    return out, res


def kernel(embedding_table, node_hyperedges):
    out, _ = run(embedding_table, node_hyperedges)
    return out
